# revision 1
# baseline (speedup 1.0000x reference)
"""Binarized 2-layer conv net (BinaryConv2d -> BinaryTanh -> BinaryConv2d -> Scale)
for Trainium2, data-parallel over the batch dim across 8 NeuronCores.

Math (matching the reference):
    h   = conv2d(x, sign(w1), pad=1) + sign(b1)
    h   = sign(h)                       # sign(clip(h,-1,1)) == sign(h)
    out = (conv2d(h, sign(w2), pad=1) + sign(b2)) * scale

Device mapping (per core, 8 images):
  * x is split on the host into fp16 hi + fp16 lo (together ~24 mantissa bits,
    i.e. effectively fp32-exact), pre-padded to 66x66.
  * conv1: dy taps packed into SBUF partitions (dy0/dy1/dy2 -> 3 blocks of
    32) - dy1 slab loaded from HBM, dy0/dy2 derived by shifted on-chip
    copies; dx taps as free-dim offsets; K=96 matmuls on the two PE column
    tiles, 3 dx x 2 precisions accumulating into one PSUM bank per
    round of two 8-row pixel blocks.
  * h staging layout H: [128 partitions = 2 row-groups x 64 channels,
    4 slots x (10 rows x 66 cols)] with halo rows + zero borders, so
    sign(conv1+b1) evacuates PSUM in a single full-lane ScalarE op and
    conv2 reads all 9 taps directly with shifted access patterns.
    With c2f8: h is fp8(e4m3), each slot padded to 2x660 holding slab A
    (rows+halos) and slab B (A shifted one row), enabling DoubleRow
    matmuls that pair taps (0,dx)+(1,dx) via the k-tile stride 660.
  * conv2: 64x64 PE tiling, 4 independent tiles (2 row-groups x 2 column
    tiles) process 4 pixel blocks concurrently, K=64; 9 tap-matmuls
    (bf16) or 3 DoubleRow + 3 plain (fp8) each.
  * Output written as bf16: conv2+sign(b2) is an odd integer |v|<=577,
    P(|v|>512) ~ 0, so bf16 is exact; the final *scale happens on host.
"""

import numpy as np
import ml_dtypes

import concourse.bass as bass
import concourse.mybir as mybir
import concourse.tile as tile
from concourse import bacc
from concourse.bass_utils import run_bass_kernel_spmd

F32 = mybir.dt.float32
F16 = mybir.dt.float16
BF16 = mybir.dt.bfloat16
F8E4 = mybir.dt.float8e4

N_CORES = 8
IMGS_PER_CORE = 8
CIN, COUT = 32, 64
H = W = 64
WP = 66                    # padded width
T1_FREE = H * WP           # 4224, one dy-block slab
SLAB = 10 * WP             # 660: 8-row block + 2 halo rows, padded width
ACT_SIGN = mybir.ActivationFunctionType.Sign
ACT_IDENT = mybir.ActivationFunctionType.Identity
ALU_ADD = mybir.AluOpType.add
ALU_MULT = mybir.AluOpType.mult
DR = mybir.MatmulPerfMode.DoubleRow


def build_nc(reps: int = 1, nbuf: int = 2, pa_bufs: int = 2,
             s2s_t1: bool = True, c2f8: bool = False) -> bacc.Bacc:
    nc = bacc.Bacc("TRN2", target_bir_lowering=False)

    h_dt = F8E4 if c2f8 else BF16
    kt = 2 if c2f8 else 1          # k-tile copies of each h slot
    h_free = 4 * kt * SLAB

    xhi_t = nc.dram_tensor("xhi", [IMGS_PER_CORE, CIN, WP, WP], F16, kind="ExternalInput")
    xlo_t = nc.dram_tensor("xlo", [IMGS_PER_CORE, CIN, WP, WP], F16, kind="ExternalInput")
    w1s_t = nc.dram_tensor("w1s", [128, 192], F16, kind="ExternalInput")
    w2s_t = nc.dram_tensor("w2s", [128, 832 if c2f8 else 576], h_dt, kind="ExternalInput")
    b1s_t = nc.dram_tensor("b1s", [128, 1], F32, kind="ExternalInput")
    b2s_t = nc.dram_tensor("b2s", [128, 1], F32, kind="ExternalInput")
    out_t = nc.dram_tensor("out", [IMGS_PER_CORE, COUT, H, W], BF16, kind="ExternalOutput")

    xhi = xhi_t.ap().rearrange("n c h w -> n c (h w)")
    xlo = xlo_t.ap().rearrange("n c h w -> n c (h w)")
    outr = out_t.ap().rearrange("n o h w -> n o (h w)")

    with tile.TileContext(nc) as tc:
        # ---- persistent SBUF tensors ----
        w1 = nc.alloc_sbuf_tensor("w1sb", [128, 192], F16).ap()
        w2 = nc.alloc_sbuf_tensor("w2sb", [128, 832 if c2f8 else 576], h_dt).ap()
        b1 = nc.alloc_sbuf_tensor("b1sb", [128, 1], F32).ap()
        b2 = nc.alloc_sbuf_tensor("b2sb", [128, 1], F32).ap()
        t1 = [
            [nc.alloc_sbuf_tensor(f"t1_{p}_{b}", [96, T1_FREE], F16).ap()
             for b in range(nbuf)]
            for p in range(2)
        ]
        hb = [nc.alloc_sbuf_tensor(f"hst_{b}", [128, h_free], h_dt).ap()
              for b in range(nbuf)]

        nc.sync.dma_start(out=w1, in_=w1s_t.ap())
        nc.sync.dma_start(out=w2, in_=w2s_t.ap())
        nc.sync.dma_start(out=b1, in_=b1s_t.ap())
        nc.sync.dma_start(out=b2, in_=b2s_t.ap())
        # H borders (padding cols / edge halo rows) must stay zero forever;
        # per-image writes only touch interiors.
        for b in range(nbuf):
            nc.gpsimd.memset(hb[b][:, :], 0.0)
        if s2s_t1:
            # dy0 slab row 0 (= x_pad row 0) and dy2 slab row 63 (= x_pad
            # row 65) are all-zero pad rows never touched by the per-image
            # shifted copies below: zero them once.
            for p in range(2):
                for b in range(nbuf):
                    nc.vector.memset(t1[p][b][0:32, 0:WP], 0.0)
                    nc.vector.memset(t1[p][b][64:96, 63 * WP:T1_FREE], 0.0)

        with tc.tile_pool(name="psA", bufs=pa_bufs, space="PSUM") as pool_a, \
             tc.tile_pool(name="psB", bufs=2, space="PSUM") as pool_cd, \
             tc.tile_pool(name="ob", bufs=4) as ob_pool:
            for img_v in range(IMGS_PER_CORE * reps):
                img = img_v % IMGS_PER_CORE
                buf = img_v % nbuf
                t1h, t1l = t1[0][buf], t1[1][buf]
                hv = hb[buf].rearrange("p (s k r w) -> p s k r w",
                                       k=kt, r=10, w=WP)

                # ---- load x (3 dy-shifted overlapping slabs per precision) ----
                if s2s_t1:
                    # load only the dy1 slab from HBM; derive dy0/dy2 by
                    # shifted on-chip copies (saves 2/3 of the x HBM reads).
                    for tt, src in ((t1h, xhi), (t1l, xlo)):
                        nc.sync.dma_start(
                            out=tt[32:64, :],
                            in_=src[img, :, WP: WP + T1_FREE])
                        nc.sync.dma_start(
                            out=tt[0:32, WP:T1_FREE],
                            in_=tt[32:64, 0:T1_FREE - WP])
                        nc.sync.dma_start(
                            out=tt[64:96, 0:T1_FREE - WP],
                            in_=tt[32:64, WP:T1_FREE])
                else:
                    for tt, src in ((t1h, xhi), (t1l, xlo)):
                        for dy in range(3):
                            nc.sync.dma_start(
                                out=tt[dy * 32:(dy + 1) * 32, :],
                                in_=src[img, :, dy * WP: dy * WP + T1_FREE],
                            )
                tvh = t1h.rearrange("p (h w) -> p h w", w=WP)
                tvl = t1l.rearrange("p (h w) -> p h w", w=WP)

                # ---- conv1: 4 rounds x (2 blocks x 2 column tiles), K=96 ----
                for r in range(4):
                    bA, bB = 2 * r, 2 * r + 1
                    pa = pool_a.tile([128, 512], F32, tag="pa")
                    n_mm = 0
                    for dx in range(3):
                        lw = w1[0:96, dx * 64:(dx + 1) * 64]
                        for tv in (tvh, tvl):
                            st = n_mm == 0
                            sp = n_mm == 5
                            nc.tensor.matmul(
                                pa[0:64, :], lw,
                                tv[0:96, 8 * bA: 8 * bA + 8, dx: dx + 64],
                                start=st, stop=sp, tile_position=(0, 0))
                            nc.tensor.matmul(
                                pa[64:128, :], lw,
                                tv[0:96, 8 * bB: 8 * bB + 8, dx: dx + 64],
                                start=st, stop=sp, tile_position=(0, 64))
                            n_mm += 1
                    # h = sign(conv1 + b1): blocks bA/bB land in H slot r of
                    # partition groups g0/g1 (same free offsets -> one op).
                    nc.scalar.activation(
                        out=hv[:, r, 0, 1:9, 1:65],
                        in_=pa[:, :].rearrange("p (a b) -> p a b", b=64),
                        func=ACT_SIGN, bias=b1[:, 0:1])
                    if c2f8:
                        # slab B = A shifted one row (same PSUM block): the
                        # DoubleRow k-tile partner holding tap dy=1.
                        nc.scalar.activation(
                            out=hv[:, r, 1, 0:8, 1:65],
                            in_=pa[:, :].rearrange("p (a b) -> p a b", b=64),
                            func=ACT_SIGN, bias=b1[:, 0:1])
                        # Dep-repair: the tile tracker drops the k-tile dim
                        # from the DR matmuls' read range, so conv2 would
                        # race the B write above. Read one B element and
                        # write an exact 0.0 into the A zero-border (which
                        # conv2's tracked range covers): ACT(B) -> DVE ->
                        # conv2 matmul ordering is then enforced.
                        nc.vector.tensor_scalar(
                            out=hv[:, r, 0, 2:3, 0:1],
                            in0=hv[:, r, 1, 0:1, 1:2],
                            scalar1=0.0, scalar2=None, op0=ALU_MULT)
                    # per-slot halo DMAs as soon as sources exist: earlier
                    # overlap, and single-slot instructions the round-0
                    # conv2 waiters can order against exactly.
                    nc.sync.dma_start(out=hv[0:64, r, 0, 9:10, 1:65],
                                      in_=hv[64:128, r, 0, 1:2, 1:65])
                    nc.sync.dma_start(out=hv[64:128, r, 0, 0:1, 1:65],
                                      in_=hv[0:64, r, 0, 8:9, 1:65])
                    if r >= 1:
                        nc.sync.dma_start(out=hv[0:64, r, 0, 0:1, 1:65],
                                          in_=hv[64:128, r - 1, 0, 8:9, 1:65])
                        nc.sync.dma_start(out=hv[64:128, r - 1, 0, 9:10, 1:65],
                                          in_=hv[0:64, r, 0, 1:2, 1:65])



                # ---- conv2 ----
                if c2f8:
                    # All-DoubleRow: DR dst must sit on partitions 0:64
                    # (s3d3_mm_valid_dst_partition), so process block PAIRS
                    # (g0 slot rr, g1 slot rr) on the two 64-row tiles, each
                    # into its own [64,512] half-bank. 6 DR matmuls cover the
                    # 9 taps: pairs {(0,dx),(1,dx)} via the B slab (k-tile
                    # stride 660) and {(2,dx), zero-weight partner}.
                    wdr = w2[0:128, 0:768].rearrange("p (i k m) -> p i k m", k=2, m=64)
                    for rr in range(4):
                        pc = pool_cd.tile([64, 512], F32, tag="pc")
                        pd = pool_cd.tile([64, 512], F32, tag="pd")
                        # start matmul: zero weights, rhs = whole B interior
                        # via a simple 2-free-dim AP. Contributes 0 but (a)
                        # zero-initializes the bank and (b) is reliably
                        # ordered after the B-slab ACT write by the tracker
                        # (the DR matmuls' k-tile APs are not).
                        nc.tensor.matmul(
                            pc[0:64, :], w2[0:64, 768:832],
                            hv[0:64, rr, 1, 0:8, 1:65],
                            start=True, stop=False,
                            tile_position=(0, 0))
                        nc.tensor.matmul(
                            pd[0:64, :], w2[64:128, 768:832],
                            hv[64:128, rr, 1, 0:8, 1:65],
                            start=True, stop=False,
                            tile_position=(64, 0))
                        if rr == 0:
                            # Round-0-only zero-weight waiters on the halo
                            # rows (the 4 batched halo DMAs): PE is in-order,
                            # so these order every later conv2 matmul too.
                            for rhs0, rhs1 in (
                                    (hv[0:64, 3, 0, 0:8, 1:65],
                                     hv[64:128, 3, 0, 0:8, 1:65]),
                                    (hv[0:64, 3, 0, 2:10, 1:65],
                                     hv[64:128, 2, 0, 2:10, 1:65])):
                                nc.tensor.matmul(
                                    pc[0:64, :], w2[0:64, 768:832], rhs0,
                                    start=False, stop=False,
                                    tile_position=(0, 0))
                                nc.tensor.matmul(
                                    pd[0:64, :], w2[64:128, 768:832], rhs1,
                                    start=False, stop=False,
                                    tile_position=(64, 0))
                        for mi in range(6):
                            st, sp = False, mi == 5
                            dx = mi % 3
                            if mi < 3:
                                r0 = 0
                            else:
                                r0 = 2
                            nc.tensor.matmul(
                                pc[0:64, :], wdr[0:64, mi, :, :],
                                hv[0:64, rr, 0:2, r0:r0 + 8, dx: dx + 64],
                                start=st, stop=sp, perf_mode=DR,
                                tile_position=(0, 0))
                            nc.tensor.matmul(
                                pd[0:64, :], wdr[64:128, mi, :, :],
                                hv[64:128, rr, 0:2, r0:r0 + 8, dx: dx + 64],
                                start=st, stop=sp, perf_mode=DR,
                                tile_position=(64, 0))
                        obc_sb = ob_pool.tile([64, 512], BF16, tag="obc")
                        obd_sb = ob_pool.tile([64, 512], BF16, tag="obd")
                        nc.scalar.activation(
                            out=obc_sb[:, :], in_=pc[:, :], func=ACT_IDENT,
                            bias=b2[0:64, 0:1])
                        nc.vector.tensor_scalar(
                            out=obd_sb[:, :], in0=pd[:, :],
                            scalar1=b2[64:128, 0:1], scalar2=None, op0=ALU_ADD)
                        # g0 slot rr = image block 2rr, g1 slot rr = 2rr+1
                        nc.sync.dma_start(
                            out=outr[img, :, (2 * rr) * 512:(2 * rr) * 512 + 512],
                            in_=obc_sb[:, :])
                        nc.sync.dma_start(
                            out=outr[img, :, (2 * rr + 1) * 512:(2 * rr + 1) * 512 + 512],
                            in_=obd_sb[:, :])
                    continue

                # bf16 path: 2 rounds x 4 blocks (4 PE tiles), K=64
                for s2 in range(2):
                    pc = pool_cd.tile([128, 512], F32, tag="pc")
                    pd = pool_cd.tile([128, 512], F32, tag="pd")
                    if True:
                        for tap in range(9):
                            dy, dx = divmod(tap, 3)
                            st = tap == 0
                            sp = tap == 8
                            lw0 = w2[0:64, tap * 64:(tap + 1) * 64]
                            lw1 = w2[64:128, tap * 64:(tap + 1) * 64]
                            nc.tensor.matmul(
                                pc[0:64, :], lw0,
                                hv[0:64, 2 * s2, 0, dy: dy + 8, dx: dx + 64],
                                start=st, stop=sp, tile_position=(0, 0))
                            nc.tensor.matmul(
                                pc[64:128, :], lw0,
                                hv[0:64, 2 * s2 + 1, 0, dy: dy + 8, dx: dx + 64],
                                start=st, stop=sp, tile_position=(0, 64))
                            nc.tensor.matmul(
                                pd[0:64, :], lw1,
                                hv[64:128, 2 * s2, 0, dy: dy + 8, dx: dx + 64],
                                start=st, stop=sp, tile_position=(64, 0))
                            nc.tensor.matmul(
                                pd[64:128, :], lw1,
                                hv[64:128, 2 * s2 + 1, 0, dy: dy + 8, dx: dx + 64],
                                start=st, stop=sp, tile_position=(64, 64))
                    # out = psum + sign(b2) (exact small integers -> bf16),
                    # split across ACT/DVE; host multiplies by scale.
                    obc_sb = ob_pool.tile([128, 512], BF16, tag="obc")
                    obd_sb = ob_pool.tile([128, 512], BF16, tag="obd")
                    nc.scalar.activation(
                        out=obc_sb[:, :], in_=pc[:, :], func=ACT_IDENT,
                        bias=b2[:, 0:1])
                    nc.vector.tensor_scalar(
                        out=obd_sb[:, :], in0=pd[:, :],
                        scalar1=b2[:, 0:1], scalar2=None, op0=ALU_ADD)
                    # g0 slots 2s2, 2s2+1 = image blocks 4s2, 4s2+2
                    # g1 slots 2s2, 2s2+1 = image blocks 4s2+1, 4s2+3
                    for half, blk in ((obc_sb[0:64, :], 4 * s2),
                                      (obc_sb[64:128, :], 4 * s2 + 2),
                                      (obd_sb[0:64, :], 4 * s2 + 1),
                                      (obd_sb[64:128, :], 4 * s2 + 3)):
                        nc.sync.dma_start(
                            out=outr[img, :, blk * 512:(blk + 1) * 512], in_=half)

    nc.compile()
    return nc


_CACHE: dict = {}


def _get_nc(scale_val: float = 0.0, reps: int = 1, **kw) -> bacc.Bacc:
    key = (reps, tuple(sorted(kw.items())))
    if key not in _CACHE:
        _CACHE[key] = build_nc(reps, **kw)
    return _CACHE[key]


def _sign(a: np.ndarray) -> np.ndarray:
    return np.where(a >= 0, np.float32(1.0), np.float32(-1.0))


def _prep_inputs(x, w1, b1, w2, b2, scale_val, c2f8=False):
    x = np.asarray(x, np.float32)
    n = x.shape[0]
    # fp16 hi/lo split (hi+lo carries ~24 mantissa bits of x)
    xhi = x.astype(np.float16)
    xlo = (x - xhi.astype(np.float32)).astype(np.float16)
    xhi_pad = np.zeros((n, CIN, WP, WP), np.float16)
    xlo_pad = np.zeros((n, CIN, WP, WP), np.float16)
    xhi_pad[:, :, 1:65, 1:65] = xhi
    xlo_pad[:, :, 1:65, 1:65] = xlo

    w1b = _sign(np.asarray(w1, np.float32))           # [64o, 32c, 3, 3]
    w2b = _sign(np.asarray(w2, np.float32))           # [64o, 64c, 3, 3]
    w1s = np.zeros((128, 192), np.float16)
    for dx in range(3):
        for dy in range(3):
            w1s[dy * 32:dy * 32 + 32, dx * 64:(dx + 1) * 64] = w1b[:, :, dy, dx].T
    h_np = ml_dtypes.float8_e4m3 if c2f8 else ml_dtypes.bfloat16
    w2s = np.zeros((128, 832 if c2f8 else 576), h_np)
    if c2f8:
        # 6 DoubleRow pairs x k-tile x 64: pair mi<3 = {tap(0,dx), tap(1,dx)},
        # pair mi>=3 = {tap(2,dx), zero partner}
        for mi in range(6):
            dx = mi % 3
            taps = [(0, dx), (1, dx)] if mi < 3 else [(2, dx), None]
            for k, tp in enumerate(taps):
                if tp is None:
                    continue
                blk = w2b[:, :, tp[0], tp[1]].T.astype(h_np)
                w2s[0:64, mi * 128 + k * 64: mi * 128 + (k + 1) * 64] = blk
                w2s[64:128, mi * 128 + k * 64: mi * 128 + (k + 1) * 64] = blk
    else:
        for dy in range(3):
            for dx in range(3):
                tap = dy * 3 + dx
                blk = w2b[:, :, dy, dx].T.astype(h_np)
                w2s[0:64, tap * 64:(tap + 1) * 64] = blk
                w2s[64:128, tap * 64:(tap + 1) * 64] = blk
    b1s = np.tile(_sign(np.asarray(b1, np.float32)), 2).reshape(128, 1).astype(np.float32)
    b2s = np.tile(_sign(np.asarray(b2, np.float32)), 2).reshape(128, 1).astype(np.float32)

    per = n // N_CORES
    in_maps = []
    for i in range(N_CORES):
        sl = slice(i * per, (i + 1) * per)
        in_maps.append({
            "xhi": np.ascontiguousarray(xhi_pad[sl]),
            "xlo": np.ascontiguousarray(xlo_pad[sl]),
            "w1s": w1s, "w2s": w2s, "b1s": b1s, "b2s": b2s,
        })
    return in_maps


def kernel(x, w1, b1, w2, b2, scale) -> np.ndarray:
    scale_val = float(np.asarray(scale).reshape(-1)[0])
    import os
    kw = {"c2f8": True}
    if os.environ.get("K_NO_C2F8"):
        kw["c2f8"] = False
    nc = _get_nc(reps=1, **kw)
    in_maps = _prep_inputs(x, w1, b1, w2, b2, scale_val, c2f8=kw["c2f8"])
    res = run_bass_kernel_spmd(nc, in_maps, core_ids=list(range(N_CORES)))
    out = np.concatenate([r["out"] for r in res.results], axis=0)
    return out.astype(np.float32) * np.float32(scale_val)


if __name__ == "__main__":
    rng = np.random.default_rng(0)
    ins = {
        "x": rng.standard_normal((64, 32, 64, 64), dtype=np.float32),
        "w1": (rng.standard_normal((64, 32, 3, 3)) * 0.05).astype(np.float32),
        "b1": (rng.standard_normal((64,)) * 0.05).astype(np.float32),
        "w2": (rng.standard_normal((64, 64, 3, 3)) * 0.05).astype(np.float32),
        "b2": (rng.standard_normal((64,)) * 0.05).astype(np.float32),
        "scale": np.array([0.001], np.float32),
    }
    out = kernel(**ins)
    print("out", out.shape, out.dtype, float(np.abs(out).mean()))

